# revision 3
# baseline (speedup 1.0000x reference)
"""Single-head causal attention block (QKV projection + attention) on 8 TRN2 cores.

Reference computation (per batch element b, batch-sharded 1 core each):
    qkv = x[b] @ W.T + b          # [T, 3E]
    q, k, v = split(qkv)          # each [T, E]
    s = (q @ k.T) / sqrt(E), causal-masked
    y = softmax(s) @ v            # [T, E]

Shapes: B=8, T=2048, E=1024.  Design notes (all HW-measured on TRN2):
  - Host-prepped layouts so no on-device transposes are needed:
      q^T, k^T computed in [E, T] layout (score matmul operands),
      v computed in [T, E] layout (PV matmul rhs),
      scores computed transposed S^T[tk, tq] so exp needs no partition reduce.
  - Inputs are loaded with ONE large DMA per tensor (x^T 4MB, Wqk 4MB, Wv
    2MB; 16-32KB contiguous per partition line, ~320-360 GB/s measured).
    The previous per-slice scheme (~170 DMAs) lost ~130us to per-DMA fixed
    costs (~2us completion latency each, FIFO per HWDGE ring).  x^T goes on
    the sync-engine ring, weights on the scalar-engine ring; no load tile
    is ever slot-reused so every DMA stays on the 2-wait DIRECT2D encoding.
  - ACT/DVE instructions carry a ~1-2us fixed cost each, so all copy-outs
    and elementwise ops are batched over wide multi-bank PSUM tiles:
    one 2048-wide activation per qkv output row-block instead of four
    512-wide ones, one exp over two score tiles, one bias-add per two v
    row-blocks, one normalization multiply per output row-block.
  - Softmax without max-subtraction: scores here are ~N(0, 0.33), so
    unnormalized exp() is numerically safe; masked entries get -50 added
    (exp -> ~2e-22).  Row sums Z come from a ones-column matmul fused into
    the PV accumulation pattern; normalization is a per-partition
    tensor_scalar multiply at the end.
  - Causal structure skips entire 128x512 score tiles above the diagonal
    and the corresponding PV accumulation terms (~2x on attention FLOPs).
  - All matmul operands are bf16 (PSUM accumulation is fp32); measured
    end-to-end error vs the fp32 reference is ~2.5e-3 relmax.
"""

import numpy as np
import ml_dtypes
from contextlib import ExitStack

import concourse.bass as bass
import concourse.bacc as bacc
import concourse.mybir as mybir
import concourse.tile as tile
from concourse.bass_utils import run_bass_kernel_spmd

FP32 = mybir.dt.float32
BF16 = mybir.dt.bfloat16
AF = mybir.ActivationFunctionType
BF16NP = ml_dtypes.bfloat16

B, T, E = 8, 2048, 1024
P = 128
NE = E // P            # 8 e-tiles (contraction)
NT = T // P            # 16 t-tiles
NC = 4                 # tq chunks of 512
CH = T // NC           # 512
SCALE = 1.0 / np.sqrt(E)
MASK_NEG = -50.0


def _build_nc(n_reps=1):
    nc = bacc.Bacc()

    xt_d = nc.declare_dram_parameter("xt", [P, NE * T], BF16, isOutput=False)
    wqk_d = nc.declare_dram_parameter("wqk", [P, 2 * NE * NE * P], BF16, isOutput=False)
    wv_d = nc.declare_dram_parameter("wv", [P, NE * E], BF16, isOutput=False)
    bqk_d = nc.declare_dram_parameter("bqk", [P, 2 * NE], FP32, isOutput=False)
    bvrep_d = nc.declare_dram_parameter("bvrep", [P, 2 * E], FP32, isOutput=False)
    masks_d = nc.declare_dram_parameter("masks", [P, 4 * CH], BF16, isOutput=False)
    y_d = nc.declare_dram_parameter("y", [T, E], FP32, isOutput=True)

    with tile.TileContext(nc) as tc:
        with ExitStack() as ctx:
            # ---- persistent pools (live through whole kernel) ----
            const_pool = ctx.enter_context(tc.tile_pool(name="const", bufs=1))
            qk_pool = ctx.enter_context(tc.tile_pool(name="qk", bufs=2 * NE))
            v_pool = ctx.enter_context(tc.tile_pool(name="v", bufs=1))

            ones_col = const_pool.tile([P, 4], BF16, tag="ones", name="ones")
            nc.vector.memset(ones_col[:], 1.0)

            qk_sb = [qk_pool.tile([P, T], BF16, tag="qk", name="qk") for _ in range(2 * NE)]
            v_all = v_pool.tile([P, NT * E], BF16, tag="v", name="v")

            # benchmark-only: run the whole body n_reps times on-device so
            # per-kernel time can be extracted from wall-clock deltas
            if n_reps > 1:
                ctx.enter_context(tc.For_i(0, n_reps, 1))

            # ---- phase 1: qkv projection ----
            with ExitStack() as p1:
                xt_pool = p1.enter_context(tc.tile_pool(name="xt", bufs=1))
                wqk_pool = p1.enter_context(tc.tile_pool(name="wqkp", bufs=1))
                wv_pool = p1.enter_context(tc.tile_pool(name="wvp", bufs=1))
                ps1 = p1.enter_context(tc.tile_pool(name="ps1", bufs=2, space="PSUM"))

                # the two big loads go on different HWDGE rings in parallel;
                # small/late tensors are queued behind them
                xt_sb = xt_pool.tile([P, NE * T], BF16, tag="xt", name="xt")
                nc.sync.dma_start(xt_sb[:], xt_d[:])
                wqk_sb = wqk_pool.tile([P, 2 * NE * NE * P], BF16, tag="wqk", name="wqk")
                nc.scalar.dma_start(wqk_sb[:], wqk_d[:])
                bqk_sb = const_pool.tile([P, 2 * NE], FP32, tag="bqk", name="bqk")
                nc.sync.dma_start(bqk_sb[:], bqk_d[:])
                bvrep = const_pool.tile([P, 2 * E], FP32, tag="bvrep", name="bvrep")
                nc.sync.dma_start(bvrep[:], bvrep_d[:])
                mask_sb = const_pool.tile([P, 4 * CH], BF16, tag="mask", name="mask")
                nc.sync.dma_start(mask_sb[:], masks_d[:])
                wv_sb = wv_pool.tile([P, NE * E], BF16, tag="wv", name="wv")
                nc.scalar.dma_start(wv_sb[:], wv_d[:])

                # q^T and k^T in [f, t] layout; one 2048-wide 4-bank PSUM
                # tile and ONE activation copy-out per f-tile
                for ft in range(2 * NE):
                    ps = ps1.tile([P, 4 * CH], FP32, tag="ps1", name="ps1")
                    for tch in range(NC):
                        for e in range(NE):
                            nc.tensor.matmul(
                                ps[:, tch * CH:(tch + 1) * CH],
                                lhsT=wqk_sb[:, (ft * NE + e) * P:(ft * NE + e + 1) * P],
                                rhs=xt_sb[:, e * T + tch * CH:e * T + (tch + 1) * CH],
                                start=(e == 0),
                                stop=(e == NE - 1),
                            )
                    # bias add + 1/sqrt(E) score scale folded into q
                    # copy-out: out = in*scale + bias (bias prescaled)
                    sc = SCALE if ft < NE else 1.0
                    nc.scalar.activation(
                        qk_sb[ft][:],
                        ps[:],
                        AF.Identity,
                        bias=bqk_sb[:, ft:ft + 1],
                        scale=sc,
                    )

                # v in [t, e] layout; two t-tiles per PSUM tile and ONE
                # bias tensor_add per pair
                for tp in range(NT // 2):
                    ps = ps1.tile([P, 4 * CH], FP32, tag="ps1", name="ps1")
                    for half in range(2):
                        tt = 2 * tp + half
                        for ec in range(2):
                            for e in range(NE):
                                nc.tensor.matmul(
                                    ps[:, (2 * half + ec) * CH:(2 * half + ec + 1) * CH],
                                    lhsT=xt_sb[:, e * T + tt * P:e * T + (tt + 1) * P],
                                    rhs=wv_sb[:, e * E + ec * CH:e * E + (ec + 1) * CH],
                                    start=(e == 0),
                                    stop=(e == NE - 1),
                                )
                    # bias varies along free dim -> tensor add of the
                    # host-replicated (x2) bias tile, writes bf16 directly
                    nc.vector.tensor_add(
                        v_all[:, 2 * tp * E:(2 * tp + 2) * E], ps[:], bvrep[:])

            # ---- phases 2+3: scores+softmax+PV, per tq chunk ----
            with ExitStack() as p2:
                exps_pool = p2.enter_context(tc.tile_pool(name="exps", bufs=10))
                y_pool = p2.enter_context(tc.tile_pool(name="yst", bufs=3))
                zr_pool = p2.enter_context(tc.tile_pool(name="zr", bufs=2))
                ps2 = p2.enter_context(tc.tile_pool(name="ps2", bufs=2, space="PSUM"))
                psy = p2.enter_context(tc.tile_pool(name="psy", bufs=2, space="PSUM"))

                for c in range(NC):
                    n_tk = (c + 1) * (CH // P)  # tk tiles at/below diagonal
                    # scores+exp in groups of two tk tiles: one 1024-wide
                    # 2-bank PSUM tile, one (optional) mask add, one exp
                    exps_tiles = []  # [P, 2*CH] tiles, one per tk pair
                    for g in range(n_tk // 2):
                        ps = ps2.tile([P, 2 * CH], FP32, tag="ps2", name="ps2")
                        for i in range(2):
                            tk = 2 * g + i
                            for e in range(NE):
                                nc.tensor.matmul(
                                    ps[:, i * CH:(i + 1) * CH],
                                    lhsT=qk_sb[NE + e][:, tk * P:(tk + 1) * P],
                                    rhs=qk_sb[e][:, c * CH:(c + 1) * CH],
                                    start=(e == 0),
                                    stop=(e == NE - 1),
                                )
                        dpair = g - 2 * c  # 0,1 for the two diagonal-crossing pairs
                        if dpair >= 0:  # additive causal mask, two tiles at once
                            nc.vector.tensor_add(
                                ps[:], ps[:],
                                mask_sb[:, dpair * 2 * CH:(dpair + 1) * 2 * CH])
                        et = exps_pool.tile([P, 2 * CH], BF16, tag="es", name="es")
                        nc.scalar.activation(et[:], ps[:], AF.Exp)
                        exps_tiles.append(et)

                    def exp_ap(tk, j):
                        # [P, P] stationary slice for (tk block, tq sub-tile j)
                        return exps_tiles[tk // 2][:, (tk % 2) * CH + j * P:
                                                   (tk % 2) * CH + (j + 1) * P]

                    # row sums Z for all four tq sub-tiles, then ONE reciprocal
                    ps_z = ps2.tile([P, 2 * CH], FP32, tag="ps2", name="ps2")
                    for j in range(CH // P):
                        nj = c * (CH // P) + j + 1
                        for tk in range(nj):
                            nc.tensor.matmul(
                                ps_z[:, 4 * j:4 * j + 4],
                                lhsT=exp_ap(tk, j),
                                rhs=ones_col[:],
                                start=(tk == 0),
                                stop=(tk == nj - 1),
                            )
                    zr = zr_pool.tile([P, 16], FP32, tag="zr", name="zr")
                    nc.vector.reciprocal(zr[:], ps_z[:, 0:16])

                    # PV accumulation; one 1024-wide PSUM tile and ONE
                    # normalization multiply per tq sub-tile
                    for j in range(CH // P):
                        tq = c * (CH // P) + j
                        nj = tq + 1
                        ps_y = psy.tile([P, 2 * CH], FP32, tag="psy", name="psy")
                        for ec in range(2):
                            for tk in range(nj):
                                nc.tensor.matmul(
                                    ps_y[:, ec * CH:(ec + 1) * CH],
                                    lhsT=exp_ap(tk, j),
                                    rhs=v_all[:, tk * E + ec * CH:tk * E + (ec + 1) * CH],
                                    start=(tk == 0),
                                    stop=(tk == nj - 1),
                                )
                        y_t = y_pool.tile([P, E], FP32, tag="y", name="y")
                        nc.vector.tensor_scalar_mul(
                            y_t[:], ps_y[:], zr[:, 4 * j:4 * j + 1])
                        nc.sync.dma_start(y_d[tq * P:(tq + 1) * P, :], y_t[:])
    nc.finalize()  # run the Bacc pass pipeline (wait splitting, reg alloc, ...)
    return nc


_NC_CACHE = {}


def _get_nc(n_reps=1):
    if n_reps not in _NC_CACHE:
        _NC_CACHE[n_reps] = _build_nc(n_reps)
    return _NC_CACHE[n_reps]


def _prep_inputs(x, W, b):
    # xt[p, a*T + t] = x[b, t, a*128+p]
    xt = np.ascontiguousarray(
        x.reshape(B, T, NE, P).transpose(0, 3, 2, 1).reshape(B, P, NE * T)
    ).astype(BF16NP)
    # wqk[p, (ft*NE+e)*P + f'] = W[ft*128+f', e*128+p]
    wqk = np.ascontiguousarray(
        W[:2 * E].reshape(2 * NE, P, NE, P).transpose(3, 0, 2, 1).reshape(P, -1)
    ).astype(BF16NP)
    # wv[p, e*E + eo] = W[2E+eo, e*128+p]
    wv = np.ascontiguousarray(
        W[2 * E:].reshape(E, NE, P).transpose(2, 1, 0).reshape(P, -1)
    ).astype(BF16NP)
    # ACT applies out = in*scale + bias, so the q bias is prescaled
    bqk = b[:2 * E].astype(np.float32).copy()
    bqk[:E] *= SCALE
    bqk = np.ascontiguousarray(bqk.reshape(2 * NE, P).T)
    bvrep = np.broadcast_to(b[2 * E:].astype(np.float32), (P, 2, E)).reshape(P, 2 * E).copy()
    ii = np.arange(P)[:, None]
    jj = np.arange(CH)[None, :]
    masks = np.concatenate(
        [np.where(jj >= d * P + ii, 0.0, MASK_NEG) for d in range(4)], axis=1
    ).astype(BF16NP)
    shared = {"wqk": wqk, "wv": wv, "bqk": bqk, "bvrep": bvrep, "masks": masks}
    return [{"xt": np.ascontiguousarray(xt[i]), **shared} for i in range(B)]


def run(x, W, b, **spmd_kwargs):
    nc = _get_nc()
    in_maps = _prep_inputs(np.asarray(x), np.asarray(W), np.asarray(b))
    res = run_bass_kernel_spmd(nc, in_maps, list(range(B)), **spmd_kwargs)
    y = np.stack([res.results[i]["y"] for i in range(B)]).astype(np.float32)
    return y, res


def kernel(x, W, b):
    y, _ = run(x, W, b)
    return y


# revision 5
# speedup vs baseline: 1.5940x; 1.5940x over previous
"""Single-head causal attention block (QKV projection + attention) on 8 TRN2 cores.

Reference computation (per batch element b, batch-sharded 1 core each):
    qkv = x[b] @ W.T + b          # [T, 3E]
    q, k, v = split(qkv)          # each [T, E]
    s = (q @ k.T) / sqrt(E), causal-masked
    y = softmax(s) @ v            # [T, E]

Shapes: B=8, T=2048, E=1024.  Design notes (all HW-measured on TRN2):
  - Host-prepped layouts so no on-device transposes are needed:
      q^T, k^T computed in [E, T] layout (score matmul operands),
      v computed in [T, E] layout (PV matmul rhs),
      scores computed transposed S^T[tk, tq] so exp needs no partition reduce.
  - fp8e4 DoubleRow matmuls (2 K-subtiles per instruction, ~1.4x measured
    over bf16, 2-4x under the cost model) carry the projection and score
    GEMMs.  Error-compensated splitting keeps accuracy well inside the
    2e-2 gate (measured 1.5e-2 end to end):
      q,k = (xh + xl) @ Wqk8        (x split into fp8 high+low halves)
      v   = xh@Wh + xh@Wl + xl@Wh   (both operands split, lo*lo dropped)
      scores = q8 @ k8              (q,k stored fp8 at natural scale; the
                                     1/sqrt(E) is folded into the exp
                                     activation scale, and the causal mask
                                     is pre-scaled by sqrt(E))
    exp tiles and v stay bf16 (PV in bf16): quantizing those to fp8 fails
    the accuracy gate.  W is pre-scaled by 256 into fp8 range; the 1/256
    comes out in the q/k activation copy-out, and for v it rides through
    the whole attention unchanged because the row-sum Z is computed with a
    256-valued ones column, so (e@v256)/(256 Z) = y.
  - Inputs are loaded with ONE large DMA per tensor (8-32KB contiguous per
    partition line, ~320-360 GB/s measured).  A per-slice scheme (~170
    DMAs) measured ~130us of per-DMA fixed costs (~2us completion latency
    each, FIFO per HWDGE ring).  x goes on the sync-engine ring, weights
    on the scalar-engine ring; no load tile is ever slot-reused, keeping
    every DMA on the 2-wait DIRECT2D encoding.
  - ACT/DVE instructions carry a ~1-2us fixed cost, so all copy-outs and
    elementwise ops are batched over wide multi-bank PSUM tiles (2048-wide
    activations, 2-tile exps, paired bias-adds, one normalization multiply
    per output row-block).
  - Softmax without max-subtraction: scores here are ~N(0, 0.33), so
    unnormalized exp() is numerically safe; masked entries get -50/SCALE
    added pre-exp (exp -> ~1e-21).
  - Causal structure skips entire 128x512 score tiles above the diagonal
    and the corresponding PV accumulation terms (~2x on attention FLOPs).
"""

import numpy as np
import ml_dtypes
from contextlib import ExitStack

import concourse.bass as bass
import concourse.bacc as bacc
import concourse.mybir as mybir
import concourse.tile as tile
from concourse.bass_utils import run_bass_kernel_spmd

FP32 = mybir.dt.float32
BF16 = mybir.dt.bfloat16
FP8 = mybir.dt.float8e4
AF = mybir.ActivationFunctionType
DR = mybir.MatmulPerfMode.DoubleRow
BF16NP = ml_dtypes.bfloat16
FP8NP = ml_dtypes.float8_e4m3

B, T, E = 8, 2048, 1024
P = 128
NE = E // P            # 8 e-tiles (contraction)
ND = NE // 2           # 4 DoubleRow pairs per full contraction
NT = T // P            # 16 t-tiles
NC = 4                 # tq chunks of 512
CH = T // NC           # 512
SCALE = 1.0 / np.sqrt(E)
MASK_NEG = -50.0
WS = 256.0             # fp8 weight pre-scale (power of two)


def _build_nc(n_reps=1):
    nc = bacc.Bacc()

    # x split into fp8 high/low parts: [pass(h,l), e, t]
    xhl_d = nc.declare_dram_parameter("xhl", [P, 2, NE, T], FP8, isOutput=False)
    wqk_d = nc.declare_dram_parameter("wqk", [P, 2 * NE, NE, P], FP8, isOutput=False)
    # v weights split into fp8 high/low parts: [half(h,l), e, eo]
    wv_d = nc.declare_dram_parameter("wv", [P, 2, NE, E], FP8, isOutput=False)
    bqk_d = nc.declare_dram_parameter("bqk", [P, 2 * NE], FP32, isOutput=False)
    bvrep_d = nc.declare_dram_parameter("bvrep", [P, 2 * E], FP32, isOutput=False)
    masks_d = nc.declare_dram_parameter("masks", [P, 4 * CH], BF16, isOutput=False)
    y_d = nc.declare_dram_parameter("y", [T, E], FP32, isOutput=True)

    with tile.TileContext(nc) as tc:
        with ExitStack() as ctx:
            # ---- persistent pools (live through whole kernel) ----
            const_pool = ctx.enter_context(tc.tile_pool(name="const", bufs=1))
            qk_pool = ctx.enter_context(tc.tile_pool(name="qk", bufs=1))
            v_pool = ctx.enter_context(tc.tile_pool(name="v", bufs=1))

            ones_col = const_pool.tile([P, 4], BF16, tag="ones", name="ones")
            nc.vector.memset(ones_col[:], WS)  # Z scaled by WS to cancel v's

            # q then k, [ft, t] f-major layout, fp8 at natural scale
            qk_sb = qk_pool.tile([P, 2 * NE, T], FP8, tag="qk", name="qk")
            v_all = v_pool.tile([P, NT * E], BF16, tag="v", name="v")

            # benchmark-only: run the whole body n_reps times on-device so
            # per-kernel time can be extracted from wall-clock deltas
            if n_reps > 1:
                ctx.enter_context(tc.For_i(0, n_reps, 1))

            # ---- phase 1: qkv projection ----
            with ExitStack() as p1:
                xt_pool = p1.enter_context(tc.tile_pool(name="xt", bufs=1))
                wqk_pool = p1.enter_context(tc.tile_pool(name="wqkp", bufs=1))
                wv_pool = p1.enter_context(tc.tile_pool(name="wvp", bufs=1))
                ps1 = p1.enter_context(tc.tile_pool(name="ps1", bufs=2, space="PSUM"))

                # the two big loads go on different HWDGE rings in parallel;
                # small/late tensors are queued behind them
                xhl_sb = xt_pool.tile([P, 2, NE, T], FP8, tag="xt", name="xt")
                nc.sync.dma_start(xhl_sb[:], xhl_d[:])
                wqk_sb = wqk_pool.tile([P, 2 * NE, NE, P], FP8, tag="wqk", name="wqk")
                nc.scalar.dma_start(wqk_sb[:], wqk_d[:])
                bqk_sb = const_pool.tile([P, 2 * NE], FP32, tag="bqk", name="bqk")
                nc.sync.dma_start(bqk_sb[:], bqk_d[:])
                bvrep = const_pool.tile([P, 2 * E], FP32, tag="bvrep", name="bvrep")
                nc.sync.dma_start(bvrep[:], bvrep_d[:])
                mask_sb = const_pool.tile([P, 4 * CH], BF16, tag="mask", name="mask")
                nc.sync.dma_start(mask_sb[:], masks_d[:])
                wv_sb = wv_pool.tile([P, 2, NE, E], FP8, tag="wv", name="wv")
                nc.scalar.dma_start(wv_sb[:], wv_d[:])

                # q^T and k^T: (xh + xl) @ W, two DoubleRow passes per chain;
                # one 2048-wide 4-bank PSUM tile and ONE activation per f-tile
                for ft in range(2 * NE):
                    ps = ps1.tile([P, 4 * CH], FP32, tag="ps1", name="ps1")
                    for tch in range(NC):
                        for hl in range(2):
                            for g in range(ND):
                                nc.tensor.matmul(
                                    ps[:, tch * CH:(tch + 1) * CH],
                                    lhsT=wqk_sb[:, ft, 2 * g:2 * g + 2, :],
                                    rhs=xhl_sb[:, hl, 2 * g:2 * g + 2,
                                               tch * CH:(tch + 1) * CH],
                                    start=(hl == 0 and g == 0),
                                    stop=(hl == 1 and g == ND - 1),
                                    perf_mode=DR,
                                )
                    # out = psum/WS + bias, stored fp8 at natural scale
                    nc.scalar.activation(
                        qk_sb[:, ft, :],
                        ps[:],
                        AF.Identity,
                        bias=bqk_sb[:, ft:ft + 1],
                        scale=1.0 / WS,
                    )

                # v (scaled by WS): xh@Wh + xh@Wl + xl@Wh, three DoubleRow
                # passes; two t-tiles per PSUM tile, ONE bias add per pair
                for tp in range(NT // 2):
                    ps = ps1.tile([P, 4 * CH], FP32, tag="ps1", name="ps1")
                    for half in range(2):
                        tt = 2 * tp + half
                        for ec in range(2):
                            chain = [(0, 0), (0, 1), (1, 0)]  # (x part, W part)
                            for ci, (xp, wp) in enumerate(chain):
                                for g in range(ND):
                                    nc.tensor.matmul(
                                        ps[:, (2 * half + ec) * CH:(2 * half + ec + 1) * CH],
                                        lhsT=xhl_sb[:, xp, 2 * g:2 * g + 2,
                                                    tt * P:(tt + 1) * P],
                                        rhs=wv_sb[:, wp, 2 * g:2 * g + 2,
                                                  ec * CH:(ec + 1) * CH],
                                        start=(ci == 0 and g == 0),
                                        stop=(ci == 2 and g == ND - 1),
                                        perf_mode=DR,
                                    )
                    # bias varies along free dim -> tensor add of the
                    # host-replicated (x2, xWS) bias tile, writes bf16
                    nc.vector.tensor_add(
                        v_all[:, 2 * tp * E:(2 * tp + 2) * E], ps[:], bvrep[:])

            # ---- phases 2+3: scores+softmax+PV, per tq chunk ----
            with ExitStack() as p2:
                exps_pool = p2.enter_context(tc.tile_pool(name="exps", bufs=10))
                y_pool = p2.enter_context(tc.tile_pool(name="yst", bufs=3))
                zr_pool = p2.enter_context(tc.tile_pool(name="zr", bufs=2))
                ps2 = p2.enter_context(tc.tile_pool(name="ps2", bufs=2, space="PSUM"))
                psy = p2.enter_context(tc.tile_pool(name="psy", bufs=2, space="PSUM"))

                for c in range(NC):
                    n_tk = (c + 1) * (CH // P)  # tk tiles at/below diagonal
                    # scores (fp8 DoubleRow) + exp in groups of two tk tiles:
                    # one 1024-wide PSUM tile, one mask add, one exp
                    exps_tiles = []  # [P, 2*CH] tiles, one per tk pair
                    for g2 in range(n_tk // 2):
                        ps = ps2.tile([P, 2 * CH], FP32, tag="ps2", name="ps2")
                        for i in range(2):
                            tk = 2 * g2 + i
                            for g in range(ND):
                                nc.tensor.matmul(
                                    ps[:, i * CH:(i + 1) * CH],
                                    lhsT=qk_sb[:, NE + 2 * g:NE + 2 * g + 2,
                                               tk * P:(tk + 1) * P],
                                    rhs=qk_sb[:, 2 * g:2 * g + 2,
                                              c * CH:(c + 1) * CH],
                                    start=(g == 0),
                                    stop=(g == ND - 1),
                                    perf_mode=DR,
                                )
                        dpair = g2 - 2 * c  # 0,1 for the diagonal-crossing pairs
                        if dpair >= 0:  # additive causal mask (pre-scaled by 1/SCALE)
                            nc.vector.tensor_add(
                                ps[:], ps[:],
                                mask_sb[:, dpair * 2 * CH:(dpair + 1) * 2 * CH])
                        et = exps_pool.tile([P, 2 * CH], BF16, tag="es", name="es")
                        # exp(s * 1/sqrt(E)) -- score scale folded in here
                        nc.scalar.activation(et[:], ps[:], AF.Exp, scale=SCALE)
                        exps_tiles.append(et)

                    def exp_ap(tk, j):
                        # [P, P] stationary slice for (tk block, tq sub-tile j)
                        return exps_tiles[tk // 2][:, (tk % 2) * CH + j * P:
                                                   (tk % 2) * CH + (j + 1) * P]

                    # row sums Z*WS for all four tq sub-tiles, ONE reciprocal
                    ps_z = ps2.tile([P, 2 * CH], FP32, tag="ps2", name="ps2")
                    for j in range(CH // P):
                        nj = c * (CH // P) + j + 1
                        for tk in range(nj):
                            nc.tensor.matmul(
                                ps_z[:, 4 * j:4 * j + 4],
                                lhsT=exp_ap(tk, j),
                                rhs=ones_col[:],
                                start=(tk == 0),
                                stop=(tk == nj - 1),
                            )
                    zr = zr_pool.tile([P, 16], FP32, tag="zr", name="zr")
                    nc.vector.reciprocal(zr[:], ps_z[:, 0:16])

                    # PV accumulation (bf16); one 1024-wide PSUM tile and ONE
                    # normalization multiply per tq sub-tile
                    for j in range(CH // P):
                        tq = c * (CH // P) + j
                        nj = tq + 1
                        ps_y = psy.tile([P, 2 * CH], FP32, tag="psy", name="psy")
                        for ec in range(2):
                            for tk in range(nj):
                                nc.tensor.matmul(
                                    ps_y[:, ec * CH:(ec + 1) * CH],
                                    lhsT=exp_ap(tk, j),
                                    rhs=v_all[:, tk * E + ec * CH:tk * E + (ec + 1) * CH],
                                    start=(tk == 0),
                                    stop=(tk == nj - 1),
                                )
                        y_t = y_pool.tile([P, E], FP32, tag="y", name="y")
                        nc.vector.tensor_scalar_mul(
                            y_t[:], ps_y[:], zr[:, 4 * j:4 * j + 1])
                        nc.sync.dma_start(y_d[tq * P:(tq + 1) * P, :], y_t[:])
    nc.finalize()  # run the Bacc pass pipeline (wait splitting, reg alloc, ...)
    return nc


_NC_CACHE = {}


def _get_nc(n_reps=1):
    if n_reps not in _NC_CACHE:
        _NC_CACHE[n_reps] = _build_nc(n_reps)
    return _NC_CACHE[n_reps]


def _prep_inputs(x, W, b):
    # x^T per batch in [e, t] layout, split into fp8 high + low parts
    xt = x.reshape(B, T, NE, P).transpose(0, 3, 2, 1)  # [B, P, NE, T] f32
    xh = xt.astype(FP8NP)
    xl = (xt - xh.astype(np.float32)).astype(FP8NP)
    xhl = np.ascontiguousarray(np.stack([xh, xl], axis=2))  # [B, P, 2, NE, T]
    # wqk[p, ft, e, f'] = W[ft*128+f', e*128+p] * WS  (fp8, single-quantized)
    wqk = np.ascontiguousarray(
        (W[:2 * E] * WS).reshape(2 * NE, P, NE, P).transpose(3, 0, 2, 1)
    ).astype(FP8NP)
    # wv[p, {h,l}, e, eo] = W[2E+eo, e*128+p] * WS split into high + low
    wvs = np.ascontiguousarray(
        (W[2 * E:] * WS).reshape(E, NE, P).transpose(2, 1, 0))  # [P, NE, E]
    wvh = wvs.astype(FP8NP)
    wvl = (wvs - wvh.astype(np.float32)).astype(FP8NP)
    wv = np.ascontiguousarray(np.stack([wvh, wvl], axis=1))  # [P, 2, NE, E]
    # q,k biases at natural scale (q/k stored natural; SCALE applied at exp)
    bqk = np.ascontiguousarray(
        b[:2 * E].astype(np.float32).reshape(2 * NE, P).T)
    # v bias, replicated x2 for the paired copy-out, scaled by WS like v
    bvrep = np.broadcast_to(
        b[2 * E:].astype(np.float32) * WS, (P, 2, E)).reshape(P, 2 * E).copy()
    ii = np.arange(P)[:, None]
    jj = np.arange(CH)[None, :]
    masks = np.concatenate(
        [np.where(jj >= d * P + ii, 0.0, MASK_NEG / SCALE) for d in range(4)],
        axis=1).astype(BF16NP)
    shared = {"wqk": wqk, "wv": wv, "bqk": bqk, "bvrep": bvrep, "masks": masks}
    return [{"xhl": np.ascontiguousarray(xhl[i]), **shared} for i in range(B)]


def run(x, W, b, **spmd_kwargs):
    nc = _get_nc()
    in_maps = _prep_inputs(np.asarray(x), np.asarray(W), np.asarray(b))
    res = run_bass_kernel_spmd(nc, in_maps, list(range(B)), **spmd_kwargs)
    y = np.stack([res.results[i]["y"] for i in range(B)]).astype(np.float32)
    return y, res


def kernel(x, W, b):
    y, _ = run(x, W, b)
    return y
